# revision 39
# baseline (speedup 1.0000x reference)
"""AffinityLoss (kernel_size=3) on 8 Trainium2 NeuronCores.

Math: with p = sigmoid(z), y in {0,1}, the BCE-of-affinity term for a pixel
pair (u, v) reduces to
    log(arg) = sp(zh_u + zh_v) - sp(zh_u) - sp(zh_v),
where zh = (2y-1)*z and sp(x) = softplus(x) = ln(1+e^x).  The 9x9xL einsum
collapses into 25 relative displacements d with separable integer border
weights; folding d/-d leaves 12 off-diagonal displacement passes + the
diagonal + one per-pixel pass.

Device kernel (data-parallel, half image per core): e = exp(zh) once (bf16; a
1-element-shifted copy keeps odd column displacements 4B-aligned for the DVE
2x mode), then per displacement one DVE shifted product e_u*e_v and an ACT
Ln(prod+1); displacements with equal interior weight share one batched Ln
with a fused per-partition accumulator (exp and ln share one ACT table set).
Border-weight deviations live only at global rows/cols {0,1,510,511}; those T
slices are staged contiguously on-chip and shipped in dense DMAs; the host
applies exact float64 corrections.
"""
import os

import numpy as np

H = W = 512
OH = OW = 510
RB, QB = 16, 8            # row-blocks x col-blocks = 128 partitions
RL, CL = 18, 68           # rows/cols per chunk including halo
ROWS_OWN, COLS_OWN = 16, 64
# displacement classes grouped by equal interior weight (3-|di|)*(3-|dj|)
CLASSES = [
    [(0, 1), (1, 0)],                     # weight 6
    [(0, 2), (2, 0)],                     # weight 3
    [(1, 1), (1, -1)],                    # weight 4
    [(1, 2), (1, -2), (2, 1), (2, -1)],   # weight 2
    [(2, 2), (2, -2)],                    # weight 1
]
D12 = [d for cls in CLASSES for d in cls]
NSLAB = 14                # slabs 0..11 = D12, 12 = pixel sp(zh), 13 = diag sp(2 zh)
NACC = 7                  # 5 classes + diag(5) + pixel(6)
N_CORES = 8

_STATE = {}


def _cr_vec(di):
    r = np.arange(H)
    cnt = np.zeros(H, dtype=np.int64)
    for ia in range(max(0, -di), min(2, 2 - di) + 1):
        cnt += ((r - ia >= 0) & (r - ia <= OH - 1)).astype(np.int64)
    return cnt


def _single_act_table_root():
    """Build an act-table root with only natural_log_exp_and_others so the
    compiler lowers both Exp and Ln from ONE table set (saves a ~1.3us
    mid-kernel ACT_TABLE_LOAD).  Returns the act_info.json path or None."""
    import json
    import shutil
    import tempfile

    try:
        from neuronxcc.driver.Job import Job
        from neuronxcc.driver.jobs.support.FindActInfo import findActInfoFile
        src_json = findActInfoFile(Job.getPackageDir(), "gen3")
    except Exception:
        return None
    src_dir = os.path.dirname(src_json)
    d = json.load(open(src_json))
    keep = [s for s in d["act_func_sets"]
            if s["name"] == "natural_log_exp_and_others"]
    if not keep:
        return None
    root = os.path.join(tempfile.gettempdir(), "affinity_act_root")
    os.makedirs(root, exist_ok=True)
    out = dict(d)
    out["act_func_sets"] = keep
    for s in keep:
        for k in d.get("pwp_file_keys", ("bkt_bin", "ctrl_bin", "profile_json")):
            sp = os.path.join(src_dir, s[k])
            dp = os.path.join(root, s[k])
            if not os.path.exists(dp):
                shutil.copy(sp, dp)
    path = os.path.join(root, "act_info.json")
    with open(path, "w") as f:
        json.dump(out, f)
    return path


def _build_program():
    import concourse.bacc as bacc
    import concourse.mybir as mybir
    from concourse.tile import TileContext

    p = _single_act_table_root()
    if p:
        # walrus side reads the env var; the bacc pre-placement side reads
        # get_activation_tables -- both must see the same filtered list.
        os.environ["BASS_ACT_ROOT_JSON_PATH"] = p
        full = bacc.get_activation_tables("gen3")
        one = {"natural_log_exp_and_others":
               full["natural_log_exp_and_others"]}
        bacc.get_activation_tables = lambda arch: one

    f32 = mybir.dt.float32
    bf16 = mybir.dt.bfloat16
    AF = mybir.ActivationFunctionType
    ALU = mybir.AluOpType

    nc = bacc.Bacc(None, target_bir_lowering=False, name="affinity_loss")
    zc = nc.dram_tensor("zc", (128, RL, CL), bf16, kind="ExternalInput")
    yc = nc.dram_tensor("yc", (128, RL, CL), bf16, kind="ExternalInput")
    accs_d = nc.dram_tensor("accs", (128, NACC), f32, kind="ExternalOutput")
    # packed: [0:448]=cols_l (14,16,2), [448:896]=cols_r
    packed_d = nc.dram_tensor("packed", (128, 896), bf16, kind="ExternalOutput")
    rows_top_d = nc.dram_tensor("rows_top", (8, NSLAB, 2, COLS_OWN), bf16,
                                kind="ExternalOutput")
    rows_bot_d = nc.dram_tensor("rows_bot", (8, NSLAB, 2, COLS_OWN), bf16,
                                kind="ExternalOutput")

    # row slabs overlap the input DMA with prep + exp (products need rows
    # 0..18); the LAST slab is smallest since it gates the exp chain
    SLABS = [(0, 12), (12, RL)]

    with TileContext(nc) as tc:
        with tc.tile_pool(name="main", bufs=1) as main, \
             tc.tile_pool(name="work", bufs=3) as work:
            zt = main.tile([128, RL, CL], bf16)
            yt = main.tile([128, RL, CL], bf16)
            sg = main.tile([128, RL, CL], bf16)
            zh = main.tile([128, RL, CL], bf16)
            ez = main.tile([128, RL, CL], bf16)
            ezs = main.tile([128, RL, CL], bf16)
            for si, (r0, r1) in enumerate(SLABS):
                # z and y on different issuing engines -> parallel DMA queues
                eng_a = nc.sync if si % 2 == 0 else nc.scalar
                eng_b = nc.scalar if si % 2 == 0 else nc.sync
                eng_a.dma_start(out=zt[:, r0:r1], in_=zc[:, r0:r1, :])
                eng_b.dma_start(out=yt[:, r0:r1], in_=yc[:, r0:r1, :])
                nc.gpsimd.tensor_scalar(sg[:, r0:r1], yt[:, r0:r1], 2.0, -1.0,
                                        ALU.mult, ALU.add)
                nc.vector.tensor_tensor(zh[:, r0:r1], sg[:, r0:r1],
                                        zt[:, r0:r1], ALU.mult)
                nc.scalar.activation(ez[:, r0:r1], zh[:, r0:r1], AF.Exp)

            owned = ez[:, 0:ROWS_OWN, 2:2 + COLS_OWN]
            T_all = main.tile([128, NSLAB, ROWS_OWN, COLS_OWN], bf16)
            accs = main.tile([128, NACC], f32)

            def emit_class(ci):
                cls = CLASSES[ci]
                slab = sum(len(CLASSES[j]) for j in range(ci))
                nd = len(cls)
                buf = work.tile([128, nd, ROWS_OWN, COLS_OWN], bf16,
                                name="clsbuf", tag=f"cls{nd}")
                for j, (di, dj) in enumerate(cls):
                    if dj % 2 == 0:
                        sh = ez[:, di:di + ROWS_OWN, 2 + dj:2 + dj + COLS_OWN]
                    else:
                        c0 = 1 + dj  # ezs[c] = ez[c+1]; even offset
                        sh = ezs[:, di:di + ROWS_OWN, c0:c0 + COLS_OWN]
                    nc.vector.tensor_tensor(buf[:, j], owned, sh, ALU.mult)
                nc.scalar.activation(T_all[:, slab:slab + nd], buf[:], AF.Ln,
                                     bias=1.0, accum_out=accs[:, ci:ci + 1])

            # staging views: edge-column values packed contiguously (a
            # direct strided DMA of 8-byte rows costs ~124us)
            stage = main.tile([128, 896], bf16)
            cl_view = stage[:, 0:448].rearrange("p (a b c) -> p a b c",
                                                a=NSLAB, b=ROWS_OWN)
            cr_view = stage[:, 448:896].rearrange("p (a b c) -> p a b c",
                                                  a=NSLAB, b=ROWS_OWN)

            # even-shift classes + diag + pixel first: they don't need ezs,
            # so the ACT pipeline keeps running while DVE builds ezs
            nc.scalar.activation(T_all[:, 12], owned, AF.Ln, bias=1.0,
                                 accum_out=accs[:, 6:7])
            emit_class(1)   # (0,2),(2,0)
            dbuf = work.tile([128, ROWS_OWN, COLS_OWN], bf16, name="dbuf")
            nc.vector.tensor_tensor(dbuf[:], owned, owned, ALU.mult)
            nc.scalar.activation(T_all[:, 13], dbuf[:], AF.Ln, bias=1.0,
                                 accum_out=accs[:, 5:6])
            emit_class(4)   # (2,2),(2,-2)
            # shifted copy for odd-dj alignment, on DVE (GPSIMD copies here
            # contend with DVE SBUF ports and slow the products ~3x)
            nc.vector.tensor_copy(ezs[:, :, 0:CL - 1], ez[:, :, 1:CL])
            emit_class(0)   # (0,1),(1,0)
            emit_class(3)   # (1,2),(1,-2),(2,1),(2,-1)
            # stage already-finished slabs (0-1, 6-13) under c2's LN
            nc.vector.tensor_copy(cl_view[:, 0:2], T_all[:, 0:2, :, 0:2])
            nc.vector.tensor_copy(cr_view[:, 0:2], T_all[:, 0:2, :, 62:64])
            nc.vector.tensor_copy(cl_view[:, 6:14], T_all[:, 6:14, :, 0:2])
            nc.vector.tensor_copy(cr_view[:, 6:14], T_all[:, 6:14, :, 62:64])
            emit_class(2)   # (1,1),(1,-1) -- last, smallest tail
            nc.scalar.dma_start(out=accs_d[:, :], in_=accs[:])
            nc.vector.tensor_copy(cl_view[:, 2:6], T_all[:, 2:6, :, 0:2])
            nc.vector.tensor_copy(cr_view[:, 2:6], T_all[:, 2:6, :, 62:64])

            # outputs split across the two HWDGE issuing engines
            nc.sync.dma_start(out=rows_top_d[:, :, :, :],
                              in_=T_all[0:8, :, 0:2, :])
            nc.scalar.dma_start(out=rows_bot_d[:, :, :, :],
                                in_=T_all[120:128, :, 14:16, :])
            nc.sync.dma_start(out=packed_d[:, :], in_=stage[:])
    nc.compile()
    return nc


def _shard_core(x, core):
    """x: (512,512) float32 -> (128, RL, CL) chunked/halo'd/zero-padded bf16."""
    import ml_dtypes
    half = core % 2
    R0 = 256 * half
    zp = np.zeros((RL * RB + 2, W + 4), dtype=np.float32)
    rows_avail = min(258, H - R0)
    zp[:rows_avail, 2:2 + W] = x[R0:R0 + rows_avail]
    r_idx = 16 * np.arange(RB)[:, None] + np.arange(RL)[None, :]
    c_idx = 64 * np.arange(QB)[:, None] + np.arange(CL)[None, :]
    out = zp[r_idx[:, None, :, None], c_idx[None, :, None, :]]  # (RB,QB,RL,CL)
    return np.ascontiguousarray(
        out.reshape(128, RL, CL).astype(ml_dtypes.bfloat16))


def _weighted_total(wr_full, wc_full, core, S_raw, rowsum, colsum, tval):
    half = core % 2
    R0 = 256 * half
    rows = np.arange(R0, R0 + 256)
    c_r = wr_full[256]
    c_c = wc_full[256]
    dev_r = rows[wr_full[rows] != c_r]
    dev_c = np.arange(W)[wc_full != c_c]
    tot = float(c_r) * float(c_c) * S_raw
    for r in dev_r:
        tot += (wr_full[r] - c_r) * c_c * rowsum[r]
    for s in dev_c:
        tot += c_r * (wc_full[s] - c_c) * colsum[s]
    for r in dev_r:
        for s in dev_c:
            tot += (wr_full[r] - c_r) * (wc_full[s] - c_c) * tval[(r, s)]
    return tot


def _host_reduce(per_core, CR):
    A1 = sum(CR[di] for di in range(-2, 3)).astype(np.float64)

    def get_sums(core, slab):
        """rowsum/colsum/tval correction data for one T slab (no raw sum)."""
        res = per_core[core]
        half = core % 2
        rowsum, tval = {}, {}
        if half == 0:
            src, row_ids = res["rows_top"], (0, 1)
        else:
            src, row_ids = res["rows_bot"], (510, 511)
        for j, r in enumerate(row_ids):
            vals = src[:, slab, j, :]  # (8 q, 64)
            rowsum[r] = vals.astype(np.float64).sum()
            for s in (0, 1):
                tval[(r, s)] = float(vals[0, s])
            for s in (510, 511):
                tval[(r, s)] = float(vals[7, s - 448])
        colsum = {}
        packed = res["packed"]
        cols_l = packed[:, 0:448].reshape(128, NSLAB, ROWS_OWN, 2)
        cols_r = packed[:, 448:896].reshape(128, NSLAB, ROWS_OWN, 2)
        for j, s in enumerate((0, 1)):
            colsum[s] = cols_l[0::8, slab, :, j].astype(np.float64).sum()
        for j, s in enumerate((510, 511)):
            colsum[s] = cols_r[7::8, slab, :, j].astype(np.float64).sum()
        return rowsum, colsum, tval

    total = 0.0
    for core in range(N_CORES):
        accs = per_core[core]["accs"].astype(np.float64)
        slab = 0
        for ci, cls in enumerate(CLASSES):
            w_int = CR[cls[0][0]][256] * CR[cls[0][1]][256]
            total += 2.0 * w_int * accs[:, ci].sum()
            for (di, dj) in cls:
                total += 2.0 * _weighted_total(CR[di], CR[dj], core, 0.0,
                                               *get_sums(core, slab))
                slab += 1
        # diag: acc col 5, slab 13, weight CR0 x CR0, x1
        total += CR[0][256] ** 2 * accs[:, 5].sum()
        total += _weighted_total(CR[0], CR[0], core, 0.0, *get_sums(core, 13))
        # pixel: acc col 6, slab 12, weight -2 * A1 x A1
        total -= 2.0 * (A1[256] ** 2 * accs[:, 6].sum()
                        + _weighted_total(A1, A1, core, 0.0,
                                          *get_sums(core, 12)))
    return total


def kernel(logits, labels):
    from concourse.bass_utils import run_bass_kernel_spmd

    if "nc" not in _STATE:
        _STATE["nc"] = _build_program()
        _STATE["CR"] = {di: _cr_vec(di).astype(np.float64) for di in range(-2, 3)}
    nc = _STATE["nc"]
    CR = _STATE["CR"]

    z = np.asarray(logits, dtype=np.float32).reshape(4, H, W)
    y = np.asarray(labels, dtype=np.float32).reshape(4, H, W)

    in_maps = []
    for core in range(N_CORES):
        img = core // 2
        in_maps.append({
            "zc": _shard_core(z[img], core),
            "yc": _shard_core(y[img], core),
        })

    res = None
    for attempt in range(3):
        try:
            res = run_bass_kernel_spmd(nc, in_maps,
                                       core_ids=list(range(N_CORES)))
            break
        except Exception:
            if attempt == 2:
                raise
            import time
            time.sleep(2.0)
    _STATE["last_results"] = res

    total = _host_reduce(res.results, CR)
    denom = 4 * 81 * OH * OW
    loss = -total / denom
    return np.float32(loss)


# revision 41
# speedup vs baseline: 1.0127x; 1.0127x over previous
"""AffinityLoss (kernel_size=3) on 8 Trainium2 NeuronCores.

Math: with p = sigmoid(z), y in {0,1}, the BCE-of-affinity term for a pixel
pair (u, v) reduces to
    log(arg) = sp(zh_u + zh_v) - sp(zh_u) - sp(zh_v),
where zh = (2y-1)*z and sp(x) = softplus(x) = ln(1+e^x).  The 9x9xL einsum
collapses into 25 relative displacements d with separable integer border
weights; folding d/-d leaves 12 off-diagonal displacement passes + the
diagonal + one per-pixel pass.

Device kernel (data-parallel, half image per core): e = exp(zh) once (bf16; a
1-element-shifted copy keeps odd column displacements 4B-aligned for the DVE
2x mode), then per displacement one DVE shifted product e_u*e_v and an ACT
Ln(prod+1); displacements with equal interior weight share one batched Ln
with a fused per-partition accumulator (exp and ln share one ACT table set).
Border-weight deviations live only at global rows/cols {0,1,510,511}; those T
slices are staged contiguously on-chip and shipped in dense DMAs; the host
applies exact float64 corrections.
"""
import os

import numpy as np

H = W = 512
OH = OW = 510
RB, QB = 16, 8            # row-blocks x col-blocks = 128 partitions
RL, CL = 18, 68           # rows/cols per chunk including halo
ROWS_OWN, COLS_OWN = 16, 64
# displacement classes grouped by equal interior weight (3-|di|)*(3-|dj|)
CLASSES = [
    [(0, 1), (1, 0)],                     # weight 6
    [(0, 2), (2, 0)],                     # weight 3
    [(1, 1), (1, -1)],                    # weight 4
    [(1, 2), (1, -2), (2, 1), (2, -1)],   # weight 2
    [(2, 2), (2, -2)],                    # weight 1
]
D12 = [d for cls in CLASSES for d in cls]
NSLAB = 14                # slabs 0..11 = D12, 12 = pixel sp(zh), 13 = diag sp(2 zh)
NACC = 7                  # 5 classes + diag(5) + pixel(6)
N_CORES = 8

_STATE = {}


def _cr_vec(di):
    r = np.arange(H)
    cnt = np.zeros(H, dtype=np.int64)
    for ia in range(max(0, -di), min(2, 2 - di) + 1):
        cnt += ((r - ia >= 0) & (r - ia <= OH - 1)).astype(np.int64)
    return cnt


def _single_act_table_root():
    """Build an act-table root with natural_log_exp_and_others moved FIRST so
    the greedy table-set pre-placement lowers both Exp and Ln from ONE set
    (saves a ~1.3us mid-kernel ACT_TABLE_LOAD).  All sets are kept (only
    reordered) so other kernels compiled in this process stay valid.
    Returns the act_info.json path or None."""
    import json
    import shutil
    import tempfile

    try:
        from neuronxcc.driver.Job import Job
        from neuronxcc.driver.jobs.support.FindActInfo import findActInfoFile
        src_json = findActInfoFile(Job.getPackageDir(), "gen3")
    except Exception:
        return None
    src_dir = os.path.dirname(src_json)
    d = json.load(open(src_json))
    first = [s for s in d["act_func_sets"]
             if s["name"] == "natural_log_exp_and_others"]
    rest = [s for s in d["act_func_sets"]
            if s["name"] != "natural_log_exp_and_others"]
    if not first:
        return None
    root = os.path.join(tempfile.gettempdir(), "affinity_act_root")
    os.makedirs(root, exist_ok=True)
    out = dict(d)
    out["act_func_sets"] = first + rest
    for s in out["act_func_sets"]:
        for k in d.get("pwp_file_keys", ("bkt_bin", "ctrl_bin", "profile_json")):
            sp = os.path.join(src_dir, s[k])
            dp = os.path.join(root, s[k])
            if not os.path.exists(dp):
                shutil.copy(sp, dp)
    path = os.path.join(root, "act_info.json")
    with open(path, "w") as f:
        json.dump(out, f)
    return path


def _build_program():
    import concourse.bacc as bacc
    import concourse.mybir as mybir
    from concourse.tile import TileContext

    p = _single_act_table_root()
    if p:
        # walrus side reads the env var; the bacc pre-placement side reads
        # get_activation_tables -- both must see the same reordered list.
        os.environ["BASS_ACT_ROOT_JSON_PATH"] = p
        full = bacc.get_activation_tables("gen3")
        key = "natural_log_exp_and_others"
        reordered = {key: full[key]}
        reordered.update((k, v) for k, v in full.items() if k != key)
        bacc.get_activation_tables = lambda arch: reordered

    f32 = mybir.dt.float32
    bf16 = mybir.dt.bfloat16
    AF = mybir.ActivationFunctionType
    ALU = mybir.AluOpType

    nc = bacc.Bacc(None, target_bir_lowering=False, name="affinity_loss")
    zc = nc.dram_tensor("zc", (128, RL, CL), bf16, kind="ExternalInput")
    yc = nc.dram_tensor("yc", (128, RL, CL), bf16, kind="ExternalInput")
    accs_d = nc.dram_tensor("accs", (128, NACC), f32, kind="ExternalOutput")
    # packed: [0:448]=cols_l (14,16,2), [448:896]=cols_r
    packed_d = nc.dram_tensor("packed", (128, 896), bf16, kind="ExternalOutput")
    rows_top_d = nc.dram_tensor("rows_top", (8, NSLAB, 2, COLS_OWN), bf16,
                                kind="ExternalOutput")
    rows_bot_d = nc.dram_tensor("rows_bot", (8, NSLAB, 2, COLS_OWN), bf16,
                                kind="ExternalOutput")

    # row slabs overlap the input DMA with prep + exp (products need rows
    # 0..18); the LAST slab is smallest since it gates the exp chain
    SLABS = [(0, 12), (12, RL)]

    with TileContext(nc) as tc:
        with tc.tile_pool(name="main", bufs=1) as main, \
             tc.tile_pool(name="work", bufs=3) as work:
            zt = main.tile([128, RL, CL], bf16)
            yt = main.tile([128, RL, CL], bf16)
            sg = main.tile([128, RL, CL], bf16)
            zh = main.tile([128, RL, CL], bf16)
            ez = main.tile([128, RL, CL], bf16)
            ezs = main.tile([128, RL, CL], bf16)
            for si, (r0, r1) in enumerate(SLABS):
                # z and y on different issuing engines -> parallel DMA queues
                eng_a = nc.sync if si % 2 == 0 else nc.scalar
                eng_b = nc.scalar if si % 2 == 0 else nc.sync
                eng_a.dma_start(out=zt[:, r0:r1], in_=zc[:, r0:r1, :])
                eng_b.dma_start(out=yt[:, r0:r1], in_=yc[:, r0:r1, :])
                nc.gpsimd.tensor_scalar(sg[:, r0:r1], yt[:, r0:r1], 2.0, -1.0,
                                        ALU.mult, ALU.add)
                nc.vector.tensor_tensor(zh[:, r0:r1], sg[:, r0:r1],
                                        zt[:, r0:r1], ALU.mult)
                nc.scalar.activation(ez[:, r0:r1], zh[:, r0:r1], AF.Exp)

            owned = ez[:, 0:ROWS_OWN, 2:2 + COLS_OWN]
            T_all = main.tile([128, NSLAB, ROWS_OWN, COLS_OWN], bf16)
            accs = main.tile([128, NACC], f32)

            def emit_class(ci):
                cls = CLASSES[ci]
                slab = sum(len(CLASSES[j]) for j in range(ci))
                nd = len(cls)
                buf = work.tile([128, nd, ROWS_OWN, COLS_OWN], bf16,
                                name="clsbuf", tag=f"cls{nd}")
                for j, (di, dj) in enumerate(cls):
                    if dj % 2 == 0:
                        sh = ez[:, di:di + ROWS_OWN, 2 + dj:2 + dj + COLS_OWN]
                    else:
                        c0 = 1 + dj  # ezs[c] = ez[c+1]; even offset
                        sh = ezs[:, di:di + ROWS_OWN, c0:c0 + COLS_OWN]
                    nc.vector.tensor_tensor(buf[:, j], owned, sh, ALU.mult)
                nc.scalar.activation(T_all[:, slab:slab + nd], buf[:], AF.Ln,
                                     bias=1.0, accum_out=accs[:, ci:ci + 1])

            # staging views: edge-column values packed contiguously (a
            # direct strided DMA of 8-byte rows costs ~124us)
            stage = main.tile([128, 896], bf16)
            cl_view = stage[:, 0:448].rearrange("p (a b c) -> p a b c",
                                                a=NSLAB, b=ROWS_OWN)
            cr_view = stage[:, 448:896].rearrange("p (a b c) -> p a b c",
                                                  a=NSLAB, b=ROWS_OWN)

            # even-shift classes + diag + pixel first: they don't need ezs,
            # so the ACT pipeline keeps running while DVE builds ezs
            nc.scalar.activation(T_all[:, 12], owned, AF.Ln, bias=1.0,
                                 accum_out=accs[:, 6:7])
            emit_class(1)   # (0,2),(2,0)
            dbuf = work.tile([128, ROWS_OWN, COLS_OWN], bf16, name="dbuf")
            nc.vector.tensor_tensor(dbuf[:], owned, owned, ALU.mult)
            nc.scalar.activation(T_all[:, 13], dbuf[:], AF.Ln, bias=1.0,
                                 accum_out=accs[:, 5:6])
            emit_class(4)   # (2,2),(2,-2)
            # shifted copy for odd-dj alignment, on DVE (GPSIMD copies here
            # contend with DVE SBUF ports and slow the products ~3x)
            nc.vector.tensor_copy(ezs[:, :, 0:CL - 1], ez[:, :, 1:CL])
            emit_class(0)   # (0,1),(1,0)
            emit_class(3)   # (1,2),(1,-2),(2,1),(2,-1)
            # stage already-finished slabs (0-1, 6-13) under c2's LN
            nc.vector.tensor_copy(cl_view[:, 0:2], T_all[:, 0:2, :, 0:2])
            nc.vector.tensor_copy(cr_view[:, 0:2], T_all[:, 0:2, :, 62:64])
            nc.vector.tensor_copy(cl_view[:, 6:14], T_all[:, 6:14, :, 0:2])
            nc.vector.tensor_copy(cr_view[:, 6:14], T_all[:, 6:14, :, 62:64])
            emit_class(2)   # (1,1),(1,-1) -- last, smallest tail
            nc.scalar.dma_start(out=accs_d[:, :], in_=accs[:])
            nc.vector.tensor_copy(cl_view[:, 2:6], T_all[:, 2:6, :, 0:2])
            nc.vector.tensor_copy(cr_view[:, 2:6], T_all[:, 2:6, :, 62:64])

            # outputs split across the two HWDGE issuing engines
            nc.sync.dma_start(out=rows_top_d[:, :, :, :],
                              in_=T_all[0:8, :, 0:2, :])
            nc.scalar.dma_start(out=rows_bot_d[:, :, :, :],
                                in_=T_all[120:128, :, 14:16, :])
            nc.sync.dma_start(out=packed_d[:, :], in_=stage[:])
    nc.compile()
    return nc


def _shard_core(x, core):
    """x: (512,512) float32 -> (128, RL, CL) chunked/halo'd/zero-padded bf16."""
    import ml_dtypes
    half = core % 2
    R0 = 256 * half
    zp = np.zeros((RL * RB + 2, W + 4), dtype=np.float32)
    rows_avail = min(258, H - R0)
    zp[:rows_avail, 2:2 + W] = x[R0:R0 + rows_avail]
    r_idx = 16 * np.arange(RB)[:, None] + np.arange(RL)[None, :]
    c_idx = 64 * np.arange(QB)[:, None] + np.arange(CL)[None, :]
    out = zp[r_idx[:, None, :, None], c_idx[None, :, None, :]]  # (RB,QB,RL,CL)
    return np.ascontiguousarray(
        out.reshape(128, RL, CL).astype(ml_dtypes.bfloat16))


def _weighted_total(wr_full, wc_full, core, S_raw, rowsum, colsum, tval):
    half = core % 2
    R0 = 256 * half
    rows = np.arange(R0, R0 + 256)
    c_r = wr_full[256]
    c_c = wc_full[256]
    dev_r = rows[wr_full[rows] != c_r]
    dev_c = np.arange(W)[wc_full != c_c]
    tot = float(c_r) * float(c_c) * S_raw
    for r in dev_r:
        tot += (wr_full[r] - c_r) * c_c * rowsum[r]
    for s in dev_c:
        tot += c_r * (wc_full[s] - c_c) * colsum[s]
    for r in dev_r:
        for s in dev_c:
            tot += (wr_full[r] - c_r) * (wc_full[s] - c_c) * tval[(r, s)]
    return tot


def _host_reduce(per_core, CR):
    A1 = sum(CR[di] for di in range(-2, 3)).astype(np.float64)

    def get_sums(core, slab):
        """rowsum/colsum/tval correction data for one T slab (no raw sum)."""
        res = per_core[core]
        half = core % 2
        rowsum, tval = {}, {}
        if half == 0:
            src, row_ids = res["rows_top"], (0, 1)
        else:
            src, row_ids = res["rows_bot"], (510, 511)
        for j, r in enumerate(row_ids):
            vals = src[:, slab, j, :]  # (8 q, 64)
            rowsum[r] = vals.astype(np.float64).sum()
            for s in (0, 1):
                tval[(r, s)] = float(vals[0, s])
            for s in (510, 511):
                tval[(r, s)] = float(vals[7, s - 448])
        colsum = {}
        packed = res["packed"]
        cols_l = packed[:, 0:448].reshape(128, NSLAB, ROWS_OWN, 2)
        cols_r = packed[:, 448:896].reshape(128, NSLAB, ROWS_OWN, 2)
        for j, s in enumerate((0, 1)):
            colsum[s] = cols_l[0::8, slab, :, j].astype(np.float64).sum()
        for j, s in enumerate((510, 511)):
            colsum[s] = cols_r[7::8, slab, :, j].astype(np.float64).sum()
        return rowsum, colsum, tval

    total = 0.0
    for core in range(N_CORES):
        accs = per_core[core]["accs"].astype(np.float64)
        slab = 0
        for ci, cls in enumerate(CLASSES):
            w_int = CR[cls[0][0]][256] * CR[cls[0][1]][256]
            total += 2.0 * w_int * accs[:, ci].sum()
            for (di, dj) in cls:
                total += 2.0 * _weighted_total(CR[di], CR[dj], core, 0.0,
                                               *get_sums(core, slab))
                slab += 1
        # diag: acc col 5, slab 13, weight CR0 x CR0, x1
        total += CR[0][256] ** 2 * accs[:, 5].sum()
        total += _weighted_total(CR[0], CR[0], core, 0.0, *get_sums(core, 13))
        # pixel: acc col 6, slab 12, weight -2 * A1 x A1
        total -= 2.0 * (A1[256] ** 2 * accs[:, 6].sum()
                        + _weighted_total(A1, A1, core, 0.0,
                                          *get_sums(core, 12)))
    return total


def kernel(logits, labels):
    from concourse.bass_utils import run_bass_kernel_spmd

    if "nc" not in _STATE:
        _STATE["nc"] = _build_program()
        _STATE["CR"] = {di: _cr_vec(di).astype(np.float64) for di in range(-2, 3)}
    nc = _STATE["nc"]
    CR = _STATE["CR"]

    z = np.asarray(logits, dtype=np.float32).reshape(4, H, W)
    y = np.asarray(labels, dtype=np.float32).reshape(4, H, W)

    in_maps = []
    for core in range(N_CORES):
        img = core // 2
        in_maps.append({
            "zc": _shard_core(z[img], core),
            "yc": _shard_core(y[img], core),
        })

    res = None
    for attempt in range(3):
        try:
            res = run_bass_kernel_spmd(nc, in_maps,
                                       core_ids=list(range(N_CORES)))
            break
        except Exception:
            if attempt == 2:
                raise
            import time
            time.sleep(2.0)
    _STATE["last_results"] = res

    total = _host_reduce(res.results, CR)
    denom = 4 * 81 * OH * OW
    loss = -total / denom
    return np.float32(loss)


# revision 42
# speedup vs baseline: 1.0195x; 1.0067x over previous
"""AffinityLoss (kernel_size=3) on 8 Trainium2 NeuronCores.

Math: with p = sigmoid(z), y in {0,1}, the BCE-of-affinity term for a pixel
pair (u, v) reduces to
    log(arg) = sp(zh_u + zh_v) - sp(zh_u) - sp(zh_v),
where zh = (2y-1)*z and sp(x) = softplus(x) = ln(1+e^x).  The 9x9xL einsum
collapses into 25 relative displacements d with separable integer border
weights; folding d/-d leaves 12 off-diagonal displacement passes + the
diagonal + one per-pixel pass.

Device kernel (data-parallel, half image per core): e = exp(zh) once (bf16; a
1-element-shifted copy keeps odd column displacements 4B-aligned for the DVE
2x mode), then per displacement one DVE shifted product e_u*e_v and an ACT
Ln(prod+1); displacements with equal interior weight share one batched Ln
with a fused per-partition accumulator (exp and ln share one ACT table set).
Border-weight deviations live only at global rows/cols {0,1,510,511}; those T
slices are staged contiguously on-chip and shipped in dense DMAs; the host
applies exact float64 corrections.
"""
import os

import numpy as np

H = W = 512
OH = OW = 510
RB, QB = 16, 8            # row-blocks x col-blocks = 128 partitions
RL, CL = 18, 68           # rows/cols per chunk including halo
ROWS_OWN, COLS_OWN = 16, 64
# displacement classes grouped by equal interior weight (3-|di|)*(3-|dj|)
CLASSES = [
    [(0, 1), (1, 0)],                     # weight 6
    [(0, 2), (2, 0)],                     # weight 3
    [(1, 1), (1, -1)],                    # weight 4
    [(1, 2), (1, -2), (2, 1), (2, -1)],   # weight 2
    [(2, 2), (2, -2)],                    # weight 1
]
D12 = [d for cls in CLASSES for d in cls]
NSLAB = 14                # slabs 0..11 = D12, 12 = pixel sp(zh), 13 = diag sp(2 zh)
NACC = 7                  # 5 classes + diag(5) + pixel(6)
N_CORES = 8

_STATE = {}


def _cr_vec(di):
    r = np.arange(H)
    cnt = np.zeros(H, dtype=np.int64)
    for ia in range(max(0, -di), min(2, 2 - di) + 1):
        cnt += ((r - ia >= 0) & (r - ia <= OH - 1)).astype(np.int64)
    return cnt


def _single_act_table_root():
    """Build an act-table root with natural_log_exp_and_others moved FIRST so
    the greedy table-set pre-placement lowers both Exp and Ln from ONE set
    (saves a ~1.3us mid-kernel ACT_TABLE_LOAD).  All sets are kept (only
    reordered) so other kernels compiled in this process stay valid.
    Returns the act_info.json path or None."""
    import json
    import shutil
    import tempfile

    try:
        from neuronxcc.driver.Job import Job
        from neuronxcc.driver.jobs.support.FindActInfo import findActInfoFile
        src_json = findActInfoFile(Job.getPackageDir(), "gen3")
    except Exception:
        return None
    src_dir = os.path.dirname(src_json)
    d = json.load(open(src_json))
    first = [s for s in d["act_func_sets"]
             if s["name"] == "natural_log_exp_and_others"]
    rest = [s for s in d["act_func_sets"]
            if s["name"] != "natural_log_exp_and_others"]
    if not first:
        return None
    root = os.path.join(tempfile.gettempdir(), "affinity_act_root")
    os.makedirs(root, exist_ok=True)
    out = dict(d)
    out["act_func_sets"] = first + rest
    for s in out["act_func_sets"]:
        for k in d.get("pwp_file_keys", ("bkt_bin", "ctrl_bin", "profile_json")):
            sp = os.path.join(src_dir, s[k])
            dp = os.path.join(root, s[k])
            if not os.path.exists(dp):
                shutil.copy(sp, dp)
    path = os.path.join(root, "act_info.json")
    with open(path, "w") as f:
        json.dump(out, f)
    return path


def _build_program():
    import concourse.bacc as bacc
    import concourse.mybir as mybir
    from concourse.tile import TileContext

    p = _single_act_table_root()
    if p:
        # walrus side reads the env var; the bacc pre-placement side reads
        # get_activation_tables -- both must see the same reordered list.
        os.environ["BASS_ACT_ROOT_JSON_PATH"] = p
        full = bacc.get_activation_tables("gen3")
        key = "natural_log_exp_and_others"
        reordered = {key: full[key]}
        reordered.update((k, v) for k, v in full.items() if k != key)
        bacc.get_activation_tables = lambda arch: reordered

    f32 = mybir.dt.float32
    bf16 = mybir.dt.bfloat16
    AF = mybir.ActivationFunctionType
    ALU = mybir.AluOpType

    nc = bacc.Bacc(None, target_bir_lowering=False, name="affinity_loss")
    zc = nc.dram_tensor("zc", (128, RL, CL), bf16, kind="ExternalInput")
    yc = nc.dram_tensor("yc", (128, RL, CL), bf16, kind="ExternalInput")
    accs_d = nc.dram_tensor("accs", (128, NACC), f32, kind="ExternalOutput")
    # packed: [0:448]=cols_l (14,16,2), [448:896]=cols_r
    packed_d = nc.dram_tensor("packed", (128, 896), bf16, kind="ExternalOutput")
    rows_top_d = nc.dram_tensor("rows_top", (8, NSLAB, 2, COLS_OWN), bf16,
                                kind="ExternalOutput")
    rows_bot_d = nc.dram_tensor("rows_bot", (8, NSLAB, 2, COLS_OWN), bf16,
                                kind="ExternalOutput")

    # row slabs overlap the input DMA with prep + exp (products need rows
    # 0..18); the LAST slab is smallest since it gates the exp chain
    SLABS = [(0, 12), (12, RL)]

    with TileContext(nc) as tc:
        with tc.tile_pool(name="main", bufs=1) as main, \
             tc.tile_pool(name="work", bufs=3) as work:
            zt = main.tile([128, RL, CL], bf16)
            yt = main.tile([128, RL, CL], bf16)
            sg = main.tile([128, RL, CL], bf16)
            zh = main.tile([128, RL, CL], bf16)
            ez = main.tile([128, RL, CL], bf16)
            ezs = main.tile([128, RL, CL], bf16)
            for si, (r0, r1) in enumerate(SLABS):
                # z and y on different issuing engines -> parallel DMA queues
                eng_a = nc.sync if si % 2 == 0 else nc.scalar
                eng_b = nc.scalar if si % 2 == 0 else nc.sync
                eng_a.dma_start(out=zt[:, r0:r1], in_=zc[:, r0:r1, :])
                eng_b.dma_start(out=yt[:, r0:r1], in_=yc[:, r0:r1, :])
                # sg+zh both on DVE: one fewer cross-engine sem hop on the
                # ramp-critical chain (DVE is idle during the ramp anyway)
                nc.vector.tensor_scalar(sg[:, r0:r1], yt[:, r0:r1], 2.0, -1.0,
                                        ALU.mult, ALU.add)
                nc.vector.tensor_tensor(zh[:, r0:r1], sg[:, r0:r1],
                                        zt[:, r0:r1], ALU.mult)
                nc.scalar.activation(ez[:, r0:r1], zh[:, r0:r1], AF.Exp)

            owned = ez[:, 0:ROWS_OWN, 2:2 + COLS_OWN]
            T_all = main.tile([128, NSLAB, ROWS_OWN, COLS_OWN], bf16)
            accs = main.tile([128, NACC], f32)

            def emit_class(ci):
                cls = CLASSES[ci]
                slab = sum(len(CLASSES[j]) for j in range(ci))
                nd = len(cls)
                buf = work.tile([128, nd, ROWS_OWN, COLS_OWN], bf16,
                                name="clsbuf", tag=f"cls{nd}")
                for j, (di, dj) in enumerate(cls):
                    if dj % 2 == 0:
                        sh = ez[:, di:di + ROWS_OWN, 2 + dj:2 + dj + COLS_OWN]
                    else:
                        c0 = 1 + dj  # ezs[c] = ez[c+1]; even offset
                        sh = ezs[:, di:di + ROWS_OWN, c0:c0 + COLS_OWN]
                    nc.vector.tensor_tensor(buf[:, j], owned, sh, ALU.mult)
                nc.scalar.activation(T_all[:, slab:slab + nd], buf[:], AF.Ln,
                                     bias=1.0, accum_out=accs[:, ci:ci + 1])

            # staging views: edge-column values packed contiguously (a
            # direct strided DMA of 8-byte rows costs ~124us)
            stage = main.tile([128, 896], bf16)
            cl_view = stage[:, 0:448].rearrange("p (a b c) -> p a b c",
                                                a=NSLAB, b=ROWS_OWN)
            cr_view = stage[:, 448:896].rearrange("p (a b c) -> p a b c",
                                                  a=NSLAB, b=ROWS_OWN)

            # even-shift classes + diag + pixel first: they don't need ezs,
            # so the ACT pipeline keeps running while DVE builds ezs
            nc.scalar.activation(T_all[:, 12], owned, AF.Ln, bias=1.0,
                                 accum_out=accs[:, 6:7])
            emit_class(1)   # (0,2),(2,0)
            dbuf = work.tile([128, ROWS_OWN, COLS_OWN], bf16, name="dbuf")
            nc.vector.tensor_tensor(dbuf[:], owned, owned, ALU.mult)
            nc.scalar.activation(T_all[:, 13], dbuf[:], AF.Ln, bias=1.0,
                                 accum_out=accs[:, 5:6])
            emit_class(4)   # (2,2),(2,-2)
            # shifted copy for odd-dj alignment, on DVE (GPSIMD copies here
            # contend with DVE SBUF ports and slow the products ~3x)
            nc.vector.tensor_copy(ezs[:, :, 0:CL - 1], ez[:, :, 1:CL])
            emit_class(0)   # (0,1),(1,0)
            emit_class(3)   # (1,2),(1,-2),(2,1),(2,-1)
            # stage already-finished slabs (0-1, 6-13) under c2's LN
            nc.vector.tensor_copy(cl_view[:, 0:2], T_all[:, 0:2, :, 0:2])
            nc.vector.tensor_copy(cr_view[:, 0:2], T_all[:, 0:2, :, 62:64])
            nc.vector.tensor_copy(cl_view[:, 6:14], T_all[:, 6:14, :, 0:2])
            nc.vector.tensor_copy(cr_view[:, 6:14], T_all[:, 6:14, :, 62:64])
            emit_class(2)   # (1,1),(1,-1) -- last, smallest tail
            nc.scalar.dma_start(out=accs_d[:, :], in_=accs[:])
            nc.vector.tensor_copy(cl_view[:, 2:6], T_all[:, 2:6, :, 0:2])
            nc.vector.tensor_copy(cr_view[:, 2:6], T_all[:, 2:6, :, 62:64])

            # outputs split across the two HWDGE issuing engines
            nc.sync.dma_start(out=rows_top_d[:, :, :, :],
                              in_=T_all[0:8, :, 0:2, :])
            nc.scalar.dma_start(out=rows_bot_d[:, :, :, :],
                                in_=T_all[120:128, :, 14:16, :])
            nc.sync.dma_start(out=packed_d[:, :], in_=stage[:])
    nc.compile()
    return nc


def _shard_core(x, core):
    """x: (512,512) float32 -> (128, RL, CL) chunked/halo'd/zero-padded bf16."""
    import ml_dtypes
    half = core % 2
    R0 = 256 * half
    zp = np.zeros((RL * RB + 2, W + 4), dtype=np.float32)
    rows_avail = min(258, H - R0)
    zp[:rows_avail, 2:2 + W] = x[R0:R0 + rows_avail]
    r_idx = 16 * np.arange(RB)[:, None] + np.arange(RL)[None, :]
    c_idx = 64 * np.arange(QB)[:, None] + np.arange(CL)[None, :]
    out = zp[r_idx[:, None, :, None], c_idx[None, :, None, :]]  # (RB,QB,RL,CL)
    return np.ascontiguousarray(
        out.reshape(128, RL, CL).astype(ml_dtypes.bfloat16))


def _weighted_total(wr_full, wc_full, core, S_raw, rowsum, colsum, tval):
    half = core % 2
    R0 = 256 * half
    rows = np.arange(R0, R0 + 256)
    c_r = wr_full[256]
    c_c = wc_full[256]
    dev_r = rows[wr_full[rows] != c_r]
    dev_c = np.arange(W)[wc_full != c_c]
    tot = float(c_r) * float(c_c) * S_raw
    for r in dev_r:
        tot += (wr_full[r] - c_r) * c_c * rowsum[r]
    for s in dev_c:
        tot += c_r * (wc_full[s] - c_c) * colsum[s]
    for r in dev_r:
        for s in dev_c:
            tot += (wr_full[r] - c_r) * (wc_full[s] - c_c) * tval[(r, s)]
    return tot


def _host_reduce(per_core, CR):
    A1 = sum(CR[di] for di in range(-2, 3)).astype(np.float64)

    def get_sums(core, slab):
        """rowsum/colsum/tval correction data for one T slab (no raw sum)."""
        res = per_core[core]
        half = core % 2
        rowsum, tval = {}, {}
        if half == 0:
            src, row_ids = res["rows_top"], (0, 1)
        else:
            src, row_ids = res["rows_bot"], (510, 511)
        for j, r in enumerate(row_ids):
            vals = src[:, slab, j, :]  # (8 q, 64)
            rowsum[r] = vals.astype(np.float64).sum()
            for s in (0, 1):
                tval[(r, s)] = float(vals[0, s])
            for s in (510, 511):
                tval[(r, s)] = float(vals[7, s - 448])
        colsum = {}
        packed = res["packed"]
        cols_l = packed[:, 0:448].reshape(128, NSLAB, ROWS_OWN, 2)
        cols_r = packed[:, 448:896].reshape(128, NSLAB, ROWS_OWN, 2)
        for j, s in enumerate((0, 1)):
            colsum[s] = cols_l[0::8, slab, :, j].astype(np.float64).sum()
        for j, s in enumerate((510, 511)):
            colsum[s] = cols_r[7::8, slab, :, j].astype(np.float64).sum()
        return rowsum, colsum, tval

    total = 0.0
    for core in range(N_CORES):
        accs = per_core[core]["accs"].astype(np.float64)
        slab = 0
        for ci, cls in enumerate(CLASSES):
            w_int = CR[cls[0][0]][256] * CR[cls[0][1]][256]
            total += 2.0 * w_int * accs[:, ci].sum()
            for (di, dj) in cls:
                total += 2.0 * _weighted_total(CR[di], CR[dj], core, 0.0,
                                               *get_sums(core, slab))
                slab += 1
        # diag: acc col 5, slab 13, weight CR0 x CR0, x1
        total += CR[0][256] ** 2 * accs[:, 5].sum()
        total += _weighted_total(CR[0], CR[0], core, 0.0, *get_sums(core, 13))
        # pixel: acc col 6, slab 12, weight -2 * A1 x A1
        total -= 2.0 * (A1[256] ** 2 * accs[:, 6].sum()
                        + _weighted_total(A1, A1, core, 0.0,
                                          *get_sums(core, 12)))
    return total


def kernel(logits, labels):
    from concourse.bass_utils import run_bass_kernel_spmd

    if "nc" not in _STATE:
        _STATE["nc"] = _build_program()
        _STATE["CR"] = {di: _cr_vec(di).astype(np.float64) for di in range(-2, 3)}
    nc = _STATE["nc"]
    CR = _STATE["CR"]

    z = np.asarray(logits, dtype=np.float32).reshape(4, H, W)
    y = np.asarray(labels, dtype=np.float32).reshape(4, H, W)

    in_maps = []
    for core in range(N_CORES):
        img = core // 2
        in_maps.append({
            "zc": _shard_core(z[img], core),
            "yc": _shard_core(y[img], core),
        })

    res = None
    for attempt in range(3):
        try:
            res = run_bass_kernel_spmd(nc, in_maps,
                                       core_ids=list(range(N_CORES)))
            break
        except Exception:
            if attempt == 2:
                raise
            import time
            time.sleep(2.0)
    _STATE["last_results"] = res

    total = _host_reduce(res.results, CR)
    denom = 4 * 81 * OH * OW
    loss = -total / denom
    return np.float32(loss)


# revision 45
# speedup vs baseline: 1.0208x; 1.0012x over previous
"""AffinityLoss (kernel_size=3) on 8 Trainium2 NeuronCores.

Math: with p = sigmoid(z), y in {0,1}, the BCE-of-affinity term for a pixel
pair (u, v) reduces to
    log(arg) = sp(zh_u + zh_v) - sp(zh_u) - sp(zh_v),
where zh = (2y-1)*z and sp(x) = softplus(x) = ln(1+e^x).  The 9x9xL einsum
collapses into 25 relative displacements d with separable integer border
weights; folding d/-d leaves 12 off-diagonal displacement passes + the
diagonal + one per-pixel pass.

Device kernel (data-parallel, half image per core): e = exp(zh) once (bf16; a
1-element-shifted copy keeps odd column displacements 4B-aligned for the DVE
2x mode), then per displacement one DVE shifted product e_u*e_v and an ACT
Ln(prod+1); displacements with equal interior weight share one batched Ln
with a fused per-partition accumulator (exp and ln share one ACT table set).
Border-weight deviations live only at global rows/cols {0,1,510,511}; those T
slices are staged contiguously on-chip and shipped in dense DMAs; the host
applies exact float64 corrections.
"""
import os

import numpy as np

H = W = 512
OH = OW = 510
RB, QB = 16, 8            # row-blocks x col-blocks = 128 partitions
RL, CL = 18, 68           # rows/cols per chunk including halo
ROWS_OWN, COLS_OWN = 16, 64
# displacement classes grouped by equal interior weight (3-|di|)*(3-|dj|)
CLASSES = [
    [(0, 1), (1, 0)],                     # weight 6
    [(0, 2), (2, 0)],                     # weight 3
    [(1, 1), (1, -1)],                    # weight 4
    [(1, 2), (1, -2), (2, 1), (2, -1)],   # weight 2
    [(2, 2), (2, -2)],                    # weight 1
]
D12 = [d for cls in CLASSES for d in cls]
NSLAB = 14                # slabs 0..11 = D12, 12 = pixel sp(zh), 13 = diag sp(2 zh)
NACC = 7                  # 5 classes + diag(5) + pixel(6)
N_CORES = 8

_STATE = {}


def _cr_vec(di):
    r = np.arange(H)
    cnt = np.zeros(H, dtype=np.int64)
    for ia in range(max(0, -di), min(2, 2 - di) + 1):
        cnt += ((r - ia >= 0) & (r - ia <= OH - 1)).astype(np.int64)
    return cnt


def _single_act_table_root():
    """Build an act-table root with natural_log_exp_and_others moved FIRST so
    the greedy table-set pre-placement lowers both Exp and Ln from ONE set
    (saves a ~1.3us mid-kernel ACT_TABLE_LOAD).  All sets are kept (only
    reordered) so other kernels compiled in this process stay valid.
    Returns the act_info.json path or None."""
    import json
    import shutil
    import tempfile

    try:
        from neuronxcc.driver.Job import Job
        from neuronxcc.driver.jobs.support.FindActInfo import findActInfoFile
        src_json = findActInfoFile(Job.getPackageDir(), "gen3")
    except Exception:
        return None
    src_dir = os.path.dirname(src_json)
    d = json.load(open(src_json))
    first = [s for s in d["act_func_sets"]
             if s["name"] == "natural_log_exp_and_others"]
    rest = [s for s in d["act_func_sets"]
            if s["name"] != "natural_log_exp_and_others"]
    if not first:
        return None
    root = os.path.join(tempfile.gettempdir(), "affinity_act_root")
    os.makedirs(root, exist_ok=True)
    out = dict(d)
    out["act_func_sets"] = first + rest
    for s in out["act_func_sets"]:
        for k in d.get("pwp_file_keys", ("bkt_bin", "ctrl_bin", "profile_json")):
            sp = os.path.join(src_dir, s[k])
            dp = os.path.join(root, s[k])
            if not os.path.exists(dp):
                shutil.copy(sp, dp)
    path = os.path.join(root, "act_info.json")
    with open(path, "w") as f:
        json.dump(out, f)
    return path


def _build_program():
    import concourse.bacc as bacc
    import concourse.mybir as mybir
    from concourse.tile import TileContext

    p = _single_act_table_root()
    if p:
        # walrus side reads the env var; the bacc pre-placement side reads
        # get_activation_tables -- both must see the same reordered list.
        os.environ["BASS_ACT_ROOT_JSON_PATH"] = p
        full = bacc.get_activation_tables("gen3")
        key = "natural_log_exp_and_others"
        reordered = {key: full[key]}
        reordered.update((k, v) for k, v in full.items() if k != key)
        bacc.get_activation_tables = lambda arch: reordered

    f32 = mybir.dt.float32
    bf16 = mybir.dt.bfloat16
    AF = mybir.ActivationFunctionType
    ALU = mybir.AluOpType

    nc = bacc.Bacc(None, target_bir_lowering=False, name="affinity_loss")
    zc = nc.dram_tensor("zc", (128, RL, CL), bf16, kind="ExternalInput")
    yc = nc.dram_tensor("yc", (128, RL, CL), bf16, kind="ExternalInput")
    accs_d = nc.dram_tensor("accs", (128, NACC), f32, kind="ExternalOutput")
    # cols_l from q=0 partitions (0::8), cols_r from q=7 partitions (7::8)
    packed_l_d = nc.dram_tensor("packed_l", (16, 448), bf16,
                                kind="ExternalOutput")
    packed_r_d = nc.dram_tensor("packed_r", (16, 448), bf16,
                                kind="ExternalOutput")
    rows_top_d = nc.dram_tensor("rows_top", (8, NSLAB, 2, COLS_OWN), bf16,
                                kind="ExternalOutput")
    rows_bot_d = nc.dram_tensor("rows_bot", (8, NSLAB, 2, COLS_OWN), bf16,
                                kind="ExternalOutput")

    # row slabs overlap the input DMA with prep + exp (products need rows
    # 0..18); the LAST slab is smallest since it gates the exp chain
    SLABS = [(0, 12), (12, RL)]

    with TileContext(nc) as tc:
        with tc.tile_pool(name="main", bufs=1) as main, \
             tc.tile_pool(name="work", bufs=3) as work:
            zt = main.tile([128, RL, CL], bf16)
            yt = main.tile([128, RL, CL], bf16)
            sg = main.tile([128, RL, CL], bf16)
            zh = main.tile([128, RL, CL], bf16)
            ez = main.tile([128, RL, CL], bf16)
            ezs = main.tile([128, RL, CL], bf16)
            for si, (r0, r1) in enumerate(SLABS):
                # z and y on different issuing engines -> parallel DMA queues
                eng_a = nc.sync if si % 2 == 0 else nc.scalar
                eng_b = nc.scalar if si % 2 == 0 else nc.sync
                eng_a.dma_start(out=zt[:, r0:r1], in_=zc[:, r0:r1, :])
                eng_b.dma_start(out=yt[:, r0:r1], in_=yc[:, r0:r1, :])
                # sg+zh both on DVE: one fewer cross-engine sem hop on the
                # ramp-critical chain (DVE is idle during the ramp anyway)
                nc.vector.tensor_scalar(sg[:, r0:r1], yt[:, r0:r1], 2.0, -1.0,
                                        ALU.mult, ALU.add)
                nc.vector.tensor_tensor(zh[:, r0:r1], sg[:, r0:r1],
                                        zt[:, r0:r1], ALU.mult)
                nc.scalar.activation(ez[:, r0:r1], zh[:, r0:r1], AF.Exp)

            owned = ez[:, 0:ROWS_OWN, 2:2 + COLS_OWN]
            T_all = main.tile([128, NSLAB, ROWS_OWN, COLS_OWN], bf16)
            accs = main.tile([128, NACC], f32)

            def emit_class(ci):
                cls = CLASSES[ci]
                slab = sum(len(CLASSES[j]) for j in range(ci))
                nd = len(cls)
                buf = work.tile([128, nd, ROWS_OWN, COLS_OWN], bf16,
                                name="clsbuf", tag=f"cls{nd}")
                for j, (di, dj) in enumerate(cls):
                    if dj % 2 == 0:
                        sh = ez[:, di:di + ROWS_OWN, 2 + dj:2 + dj + COLS_OWN]
                    else:
                        c0 = 1 + dj  # ezs[c] = ez[c+1]; even offset
                        sh = ezs[:, di:di + ROWS_OWN, c0:c0 + COLS_OWN]
                    nc.vector.tensor_tensor(buf[:, j], owned, sh, ALU.mult)
                nc.scalar.activation(T_all[:, slab:slab + nd], buf[:], AF.Ln,
                                     bias=1.0, accum_out=accs[:, ci:ci + 1])

            # staging views: edge-column values packed contiguously (a
            # direct strided DMA of 8-byte rows costs ~124us)
            stage = main.tile([128, 896], bf16)
            cl_view = stage[:, 0:448].rearrange("p (a b c) -> p a b c",
                                                a=NSLAB, b=ROWS_OWN)
            cr_view = stage[:, 448:896].rearrange("p (a b c) -> p a b c",
                                                  a=NSLAB, b=ROWS_OWN)

            # even-shift classes + diag + pixel first: they don't need ezs,
            # so the ACT pipeline keeps running while DVE builds ezs
            nc.scalar.activation(T_all[:, 12], owned, AF.Ln, bias=1.0,
                                 accum_out=accs[:, 6:7])
            emit_class(1)   # (0,2),(2,0)
            dbuf = work.tile([128, ROWS_OWN, COLS_OWN], bf16, name="dbuf")
            nc.vector.tensor_tensor(dbuf[:], owned, owned, ALU.mult)
            nc.scalar.activation(T_all[:, 13], dbuf[:], AF.Ln, bias=1.0,
                                 accum_out=accs[:, 5:6])
            emit_class(4)   # (2,2),(2,-2)
            # shifted copy for odd-dj alignment, on DVE (GPSIMD copies here
            # contend with DVE SBUF ports and slow the products ~3x)
            nc.vector.tensor_copy(ezs[:, :, 0:CL - 1], ez[:, :, 1:CL])
            emit_class(0)   # (0,1),(1,0)
            emit_class(3)   # (1,2),(1,-2),(2,1),(2,-1)
            # stage already-finished slabs (0-1, 6-13) under c2's LN
            nc.vector.tensor_copy(cl_view[:, 0:2], T_all[:, 0:2, :, 0:2])
            nc.vector.tensor_copy(cr_view[:, 0:2], T_all[:, 0:2, :, 62:64])
            nc.vector.tensor_copy(cl_view[:, 6:14], T_all[:, 6:14, :, 0:2])
            nc.vector.tensor_copy(cr_view[:, 6:14], T_all[:, 6:14, :, 62:64])
            emit_class(2)   # (1,1),(1,-1) -- last, smallest tail
            nc.scalar.dma_start(out=accs_d[:, :], in_=accs[:])
            nc.vector.tensor_copy(cl_view[:, 2:6], T_all[:, 2:6, :, 0:2])
            nc.vector.tensor_copy(cr_view[:, 2:6], T_all[:, 2:6, :, 62:64])

            # outputs split across the two HWDGE issuing engines
            nc.sync.dma_start(out=rows_top_d[:, :, :, :],
                              in_=T_all[0:8, :, 0:2, :])
            nc.scalar.dma_start(out=rows_bot_d[:, :, :, :],
                                in_=T_all[120:128, :, 14:16, :])
            # only the edge-chunk partitions are read by the host
            pgrid = stage[:].rearrange("(a b) f -> a b f", b=8)
            nc.sync.dma_start(out=packed_l_d[:, :], in_=pgrid[:, 0, 0:448])
            nc.scalar.dma_start(out=packed_r_d[:, :], in_=pgrid[:, 7, 448:896])
    nc.compile()
    return nc


def _shard_core(x, core):
    """x: (512,512) float32 -> (128, RL, CL) chunked/halo'd/zero-padded bf16."""
    import ml_dtypes
    half = core % 2
    R0 = 256 * half
    zp = np.zeros((RL * RB + 2, W + 4), dtype=np.float32)
    rows_avail = min(258, H - R0)
    zp[:rows_avail, 2:2 + W] = x[R0:R0 + rows_avail]
    r_idx = 16 * np.arange(RB)[:, None] + np.arange(RL)[None, :]
    c_idx = 64 * np.arange(QB)[:, None] + np.arange(CL)[None, :]
    out = zp[r_idx[:, None, :, None], c_idx[None, :, None, :]]  # (RB,QB,RL,CL)
    return np.ascontiguousarray(
        out.reshape(128, RL, CL).astype(ml_dtypes.bfloat16))


def _weighted_total(wr_full, wc_full, core, S_raw, rowsum, colsum, tval):
    half = core % 2
    R0 = 256 * half
    rows = np.arange(R0, R0 + 256)
    c_r = wr_full[256]
    c_c = wc_full[256]
    dev_r = rows[wr_full[rows] != c_r]
    dev_c = np.arange(W)[wc_full != c_c]
    tot = float(c_r) * float(c_c) * S_raw
    for r in dev_r:
        tot += (wr_full[r] - c_r) * c_c * rowsum[r]
    for s in dev_c:
        tot += c_r * (wc_full[s] - c_c) * colsum[s]
    for r in dev_r:
        for s in dev_c:
            tot += (wr_full[r] - c_r) * (wc_full[s] - c_c) * tval[(r, s)]
    return tot


def _host_reduce(per_core, CR):
    A1 = sum(CR[di] for di in range(-2, 3)).astype(np.float64)

    def get_sums(core, slab):
        """rowsum/colsum/tval correction data for one T slab (no raw sum)."""
        res = per_core[core]
        half = core % 2
        rowsum, tval = {}, {}
        if half == 0:
            src, row_ids = res["rows_top"], (0, 1)
        else:
            src, row_ids = res["rows_bot"], (510, 511)
        for j, r in enumerate(row_ids):
            vals = src[:, slab, j, :]  # (8 q, 64)
            rowsum[r] = vals.astype(np.float64).sum()
            for s in (0, 1):
                tval[(r, s)] = float(vals[0, s])
            for s in (510, 511):
                tval[(r, s)] = float(vals[7, s - 448])
        colsum = {}
        cols_l = res["packed_l"].reshape(16, NSLAB, ROWS_OWN, 2)
        cols_r = res["packed_r"].reshape(16, NSLAB, ROWS_OWN, 2)
        for j, s in enumerate((0, 1)):
            colsum[s] = cols_l[:, slab, :, j].astype(np.float64).sum()
        for j, s in enumerate((510, 511)):
            colsum[s] = cols_r[:, slab, :, j].astype(np.float64).sum()
        return rowsum, colsum, tval

    total = 0.0
    for core in range(N_CORES):
        accs = per_core[core]["accs"].astype(np.float64)
        slab = 0
        for ci, cls in enumerate(CLASSES):
            w_int = CR[cls[0][0]][256] * CR[cls[0][1]][256]
            total += 2.0 * w_int * accs[:, ci].sum()
            for (di, dj) in cls:
                total += 2.0 * _weighted_total(CR[di], CR[dj], core, 0.0,
                                               *get_sums(core, slab))
                slab += 1
        # diag: acc col 5, slab 13, weight CR0 x CR0, x1
        total += CR[0][256] ** 2 * accs[:, 5].sum()
        total += _weighted_total(CR[0], CR[0], core, 0.0, *get_sums(core, 13))
        # pixel: acc col 6, slab 12, weight -2 * A1 x A1
        total -= 2.0 * (A1[256] ** 2 * accs[:, 6].sum()
                        + _weighted_total(A1, A1, core, 0.0,
                                          *get_sums(core, 12)))
    return total


def kernel(logits, labels):
    from concourse.bass_utils import run_bass_kernel_spmd

    if "nc" not in _STATE:
        _STATE["nc"] = _build_program()
        _STATE["CR"] = {di: _cr_vec(di).astype(np.float64) for di in range(-2, 3)}
    nc = _STATE["nc"]
    CR = _STATE["CR"]

    z = np.asarray(logits, dtype=np.float32).reshape(4, H, W)
    y = np.asarray(labels, dtype=np.float32).reshape(4, H, W)

    in_maps = []
    for core in range(N_CORES):
        img = core // 2
        in_maps.append({
            "zc": _shard_core(z[img], core),
            "yc": _shard_core(y[img], core),
        })

    res = None
    for attempt in range(3):
        try:
            res = run_bass_kernel_spmd(nc, in_maps,
                                       core_ids=list(range(N_CORES)))
            break
        except Exception:
            if attempt == 2:
                raise
            import time
            time.sleep(2.0)
    _STATE["last_results"] = res

    total = _host_reduce(res.results, CR)
    denom = 4 * 81 * OH * OW
    loss = -total / denom
    return np.float32(loss)
